# revision 20
# baseline (speedup 1.0000x reference)
"""Trainium2 Bass kernel for nn_Attention_27977416966318 (sparse_attention).

score[b,s] = v . tanh(W @ concat(static[b,s], dynamic[b,s], dec[b]))
out = softmax(score, axis=1)

Shapes: static/dynamic [64, 2048, 256] f32, decoder_hidden [64, 256],
v [1, 768], W [768, 768].  Output [64, 2048] f32.

Strategy: data-parallel over batch B=64 across 8 NeuronCores (8 batches
per core).  W @ cat decomposes as W1 @ static + W2 @ dynamic + (W3 @ dec[b])
where the last term is a per-batch bias computed once on-device.  Inputs
are pre-transposed on the host to feature-major [256, tokens] bf16 layout
so the contraction dim lands on SBUF partitions.  Main matmuls run in
bf16 against fp32 PSUM.  tanh+bias fused on the scalar engine reading
PSUM, writing bf16.  The v-dot runs as 4 column-packed (tile_position)
M=1 matmuls that execute concurrently on disjoint 32-column strips of
the PE array.  exp() is fused into the score PSUM->SBUF copy (scores are
bounded, so the max-free softmax is numerically safe) with accum_out
producing the per-chunk softmax denominators for free.
"""

import os

import numpy as np
import ml_dtypes

import concourse.bass as bass
from concourse import bacc
import concourse.mybir as mybir
import concourse.tile as tile
from concourse.bass_utils import run_bass_kernel_spmd

B, S, H = 64, 2048, 256
H3 = 3 * H          # 768
NCORES = 8
BL = B // NCORES    # 8 batches per core
T = BL * S          # 16384 tokens per core
KT = 4              # contraction k-tiles of 128 (2 static + 2 dynamic)
MT = H3 // 128      # 6 output o-tiles
GT = 1024           # tokens per group (2 chunks of 512)
F32 = mybir.dt.float32
BF16 = mybir.dt.bfloat16
TANH = mybir.ActivationFunctionType.Tanh
EXP = mybir.ActivationFunctionType.Exp

_CACHED = {}


def build_bass():
    nc = bacc.Bacc(None, target_bir_lowering=False, debug=False)
    x = nc.dram_tensor("x_t", [2 * H, T], BF16, kind="ExternalInput")
    xr = x.rearrange("(t p) n -> p t n", p=128)  # [128, 4, T]
    dec = nc.dram_tensor("dec_t", [H, BL], BF16, kind="ExternalInput")
    wt = nc.dram_tensor("wt", [H3, H3], BF16, kind="ExternalInput")
    vv = nc.dram_tensor("v", [1, H3], F32, kind="ExternalInput")
    out = nc.dram_tensor("out", [BL, S], F32, kind="ExternalOutput")

    with tile.TileContext(nc) as tc:
        with (
            tc.tile_pool(name="const", bufs=1) as constp,
            tc.tile_pool(name="xp", bufs=2) as xp,
            tc.tile_pool(name="thp", bufs=13) as thp,
            tc.tile_pool(name="misc", bufs=1) as miscp,
            tc.tile_pool(name="hps", bufs=3, space="PSUM") as hps,
            tc.tile_pool(name="sps", bufs=2, space="PSUM") as sps,
        ):
            # ---- PE warmup: ~3.5us of zero matmuls so the HAM clock
            # gate reaches 8/8 before the real stream begins ----
            warm = constp.tile([128, 512], BF16)
            nc.vector.memset(warm, 0.0)
            warm_ps = sps.tile([128, 512], F32, tag="s", name="warm_ps")
            for i in range(18):
                nc.tensor.matmul(
                    out=warm_ps, lhsT=warm[:, 0:128], rhs=warm,
                    start=True, stop=True,
                )

            # ---- first x tiles: 4 small DMAs so the first matmul can
            # start as soon as k-tile 0 lands ----
            first_xt = xp.tile([128, KT, GT], BF16, tag="x", bufs=3, name="x_0_0")
            for kt_i in range(KT):
                nc.sync.dma_start(
                    out=first_xt[:, kt_i, :], in_=xr[:, kt_i, 0:GT]
                )

            # ---- constants on the scalar (HWDGE) queue, in parallel ----
            # wt is W.T: [k=cat-feature, o].  k-tiles 0-1 static, 2-3 dynamic,
            # 4-5 decoder.  Main-loop tiles load first, finest first.
            wtr = wt.rearrange("(t p) o -> p t o", p=128)
            wt_sb = constp.tile([128, 6, H3], BF16)
            nc.scalar.dma_start(out=wt_sb[:, 0:1, :], in_=wtr[:, 0:1, :])
            nc.scalar.dma_start(out=wt_sb[:, 1:2, :], in_=wtr[:, 1:2, :])
            nc.scalar.dma_start(out=wt_sb[:, 2:4, :], in_=wtr[:, 2:4, :])
            nc.scalar.dma_start(out=wt_sb[:, 4:6, :], in_=wtr[:, 4:6, :])
            # v in bf16 (cast during DMA): the v-dot is a bf16 matmul
            v_sb = constp.tile([128, MT], BF16)
            nc.gpsimd.dma_start(out=v_sb, in_=vv[0].rearrange("(t p) -> p t", p=128))
            dec_sb = constp.tile([128, 2, BL], BF16)
            nc.gpsimd.dma_start(
                out=dec_sb, in_=dec.rearrange("(t p) b -> p t b", p=128)
            )

            bias_sb = constp.tile([128, MT, BL], F32)

            def emit_bias():
                # bias[o, b] = sum_k W3T[k, o] dec[k, b]
                for m in range(MT):
                    bias_ps = sps.tile([128, BL], F32, tag="s", name=f"bias_ps_{m}")
                    for i in range(2):
                        nc.tensor.matmul(
                            out=bias_ps,
                            lhsT=wt_sb[:, 4 + i, m * 128 : (m + 1) * 128],
                            rhs=dec_sb[:, i, :],
                            start=(i == 0),
                            stop=(i == 1),
                        )
                    nc.vector.tensor_copy(out=bias_sb[:, m, :], in_=bias_ps)

            escores = miscp.tile([BL, S], F32)   # exp(score), filled per b
            esums = miscp.tile([BL, 4], F32)     # per-chunk exp sums

            # ---- main loop ----
            for b in range(BL):
                ths = {}

                def emit_quad(m, score_ps):
                    # column-packed v-dot: 4 chunks concurrently on col strips
                    for ci in range(4):
                        gg, cc = divmod(ci, 2)
                        nc.tensor.matmul(
                            out=score_ps[32 * ci : 32 * ci + 1, :],
                            lhsT=v_sb[:, m : m + 1],
                            rhs=ths[(gg, m)][:, cc * 512 : (cc + 1) * 512],
                            start=(m == 0),
                            stop=(m == MT - 1),
                            tile_position=(0, 32 * ci),
                        )

                score_ps = None
                for g in range(2):
                    tok0 = b * S + g * GT
                    if b == 0 and g == 0:
                        xt = first_xt
                    elif b == 0 and g == 1:
                        # early in the run HBM is still catching up; split
                        # the transfer so the first k-tiles land sooner
                        xt = xp.tile(
                            [128, KT, GT], BF16, tag="x", bufs=3, name="x_0_1"
                        )
                        nc.sync.dma_start(
                            out=xt[:, 0:2, :], in_=xr[:, 0:2, tok0 : tok0 + GT]
                        )
                        nc.sync.dma_start(
                            out=xt[:, 2:4, :], in_=xr[:, 2:4, tok0 : tok0 + GT]
                        )
                    else:
                        xt = xp.tile(
                            [128, KT, GT], BF16, tag="x", bufs=3, name=f"x_{b}_{g}"
                        )
                        nc.sync.dma_start(
                            out=xt, in_=xr[:, :, tok0 : tok0 + GT]
                        )
                    if g == 1:
                        # one PSUM bank holds the 4 chunk scores on partitions
                        # 0/32/64/96; zeroed on first use of each pool slot so
                        # the full-height exp reads finite values everywhere
                        score_ps = sps.tile(
                            [128, 512], F32, tag="s", name=f"sa_{b}"
                        )
                        nc.vector.memset(score_ps, 0.0)
                    for m in range(MT):
                        h_ps = hps.tile([128, GT], F32, tag="h", name=f"h_{b}_{g}_{m}")
                        for kt_i in range(KT):
                            for c in range(2):
                                nc.tensor.matmul(
                                    out=h_ps[:, c * 512 : (c + 1) * 512],
                                    lhsT=wt_sb[:, kt_i, m * 128 : (m + 1) * 128],
                                    rhs=xt[:, kt_i, c * 512 : (c + 1) * 512],
                                    start=(kt_i == 0),
                                    stop=(kt_i == KT - 1),
                                )
                        if b == 0 and g == 0 and m == 0:
                            # bias matmuls slot in after the first main
                            # matmul block, before the first tanh needs them
                            emit_bias()
                        if g == 1 and m >= 1:
                            # interleave score quads one m behind the main
                            # stream so they never wait on the tanh ACT
                            emit_quad(m - 1, score_ps)
                        th = thp.tile(
                            [128, GT], BF16, tag="tanh", name=f"th_{b}_{g}_{m}"
                        )
                        nc.scalar.activation(
                            out=th, in_=h_ps, func=TANH, bias=bias_sb[:, m, b : b + 1]
                        )
                        ths[(g, m)] = th
                emit_quad(MT - 1, score_ps)
                # exp fused into one full-height PSUM->SBUF copy; accum_out
                # yields the per-chunk softmax denominators for free
                stage = miscp.tile(
                    [128, 513], F32, tag="stage", bufs=2, name=f"stage_{b}"
                )
                nc.scalar.activation(
                    out=stage[:, 0:512], in_=score_ps, func=EXP,
                    accum_out=stage[:, 512:513],
                )
                # gather rows {0,32,64,96} -> escores[b] / esums[b]
                stager = stage.rearrange("(c r) f -> c r f", c=4)[:, 0, :]
                nc.gpsimd.dma_start(out=escores[b : b + 1, :], in_=stager[:, 0:512])
                nc.gpsimd.dma_start(out=esums[b : b + 1, :], in_=stager[:, 512:513])

            # ---- softmax denominator + scale ----
            sm = miscp.tile([BL, 1], F32)
            nc.vector.reduce_sum(out=sm, in_=esums, axis=mybir.AxisListType.X)
            rs = miscp.tile([BL, 1], F32)
            nc.vector.reciprocal(out=rs, in_=sm)
            ob = miscp.tile([BL, S], F32)
            nc.vector.tensor_scalar_mul(out=ob, in0=escores, scalar1=rs)
            nc.sync.dma_start(out=out[:, :], in_=ob)

    nc.compile()
    return nc


def kernel(static, dynamic, decoder_hidden, v, W):
    static = np.ascontiguousarray(np.asarray(static, dtype=np.float32))
    dynamic = np.ascontiguousarray(np.asarray(dynamic, dtype=np.float32))
    decoder_hidden = np.ascontiguousarray(np.asarray(decoder_hidden, dtype=np.float32))
    v = np.ascontiguousarray(np.asarray(v, dtype=np.float32))
    W = np.ascontiguousarray(np.asarray(W, dtype=np.float32))

    wt = np.ascontiguousarray(W.T)
    in_maps = []
    bf16 = ml_dtypes.bfloat16
    wt16 = wt.astype(bf16)
    for c in range(NCORES):
        sl = slice(c * BL, (c + 1) * BL)
        x_t = np.empty((2 * H, T), dtype=bf16)
        x_t[:H] = static[sl].reshape(T, H).T.astype(bf16)
        x_t[H:] = dynamic[sl].reshape(T, H).T.astype(bf16)
        dec_t = np.ascontiguousarray(decoder_hidden[sl].T).astype(bf16)
        in_maps.append({"x_t": x_t, "dec_t": dec_t, "wt": wt16, "v": v})

    if "nc" not in _CACHED:
        _CACHED["nc"] = build_bass()
    nc = _CACHED["nc"]

    trace = bool(int(os.environ.get("KERNEL_TRACE", "0")))
    res = run_bass_kernel_spmd(
        nc, in_maps, core_ids=list(range(NCORES)), trace=trace,
        trace_cores=list(range(NCORES)) if trace else None,
    )
    _CACHED["last_result"] = res

    out = np.concatenate([r["out"] for r in res.results], axis=0)
    return out
